# revision 1
# baseline (speedup 1.0000x reference)
"""Trainium2 Bass kernel for nn_Model_29592324670139 (dense transformer).

Sharding: 8 cores = 4 pairs. Pair b handles batch item b; within a pair the
672-token sequence (21 vars x 32 windows, window-major order) is split by
window parity (rank0 = even windows, rank1 = odd windows), 336 tokens each.
Per layer, each core projects Q/K/V for its tokens; K/V are AllGathered
within the pair; attention/FFN/LN run on local tokens. The final pooled
feature sum is AllGathered, and head+MLP run redundantly per pair.

Activations are feature-major ([d, token] with d on partitions). Matmuls in
bf16 with fp32 PSUM accumulation; softmax/LN statistics in fp32.

Self-contained: hardcodes all shapes; only needs numpy/ml_dtypes/concourse.
"""

import numpy as np
import ml_dtypes

import concourse.bass as bass
import concourse.tile as tile
from concourse import bacc, mybir
from concourse.bass import ts, ds
from concourse.bass_utils import run_bass_kernel_spmd

F32 = mybir.dt.float32
BF16 = mybir.dt.bfloat16
F32R = mybir.dt.float32r
AX = mybir.AluOpType
AF = mybir.ActivationFunctionType
XL = mybir.AxisListType

B, L, C = 4, 3072, 21
P, OUT, D, H, NL, DFF = 96, 96, 1024, 16, 2, 4096
NW = 32          # windows
SL = 336         # local tokens per core
S = 672          # full sequence
HD = 64          # head dim
NKC = D // 128   # 8 k-chunks of d_model
NFC = DFF // 128  # 32 chunks of d_ff

REPLICA_GROUPS = [[0, 1], [2, 3], [4, 5], [6, 7]]

# rank-invariant query-suffix starts per key chunk (block-causal skip)
SUF0 = [0, 105, 210, 0, 105, 210]

_BUILT = None  # cached (nc, input_names)

LAST_RESULT = None  # stash of the last BassKernelResults (for test harness)


# ----------------------------------------------------------------------------
# device program
# ----------------------------------------------------------------------------

def _build():
    nc = bacc.Bacc("TRN2", target_bir_lowering=False, debug=False,
                   enable_asserts=False, num_devices=8)

    t = {}

    def din(name, shape, dt):
        t[name] = nc.dram_tensor(name, list(shape), dt, kind="ExternalInput").ap()

    # per-core data
    din("xfull", (C, L), F32)
    din("xloc", (P, SL), BF16)
    din("maskM", (112, 6, SL), BF16)
    # embedding
    din("embW", (P, D), BF16)
    din("biases", (128, 2 * NKC + NL * (8 * NKC + NFC)), F32)  # all per-chunk vectors packed
    din("bvpack", (1, NL * D), BF16)
    for l in range(NL):
        for w in ("Wq", "Wk", "Wo", "Wv"):
            din(f"{w}{l}", (128, NKC, D), BF16)      # [p, kc, n] pre-arranged
        din(f"W1{l}", (4, 128, NKC, 1024), BF16)     # quarters [j, p, kc, n]
        din(f"W2{l}", (NKC, 128, NFC, 128), BF16)    # per-oc [oc, p, kc, c]
    din("headWs", (128, NKC, OUT), BF16)
    din("hwsumN", (1, OUT), BF16)
    din("featC", (OUT, 1), F32)
    din("c1W", (OUT, 256), BF16)
    din("c1B", (128, 2), F32)
    din("c2W", (128, 2, 64), BF16)
    din("c2B", (64, 1), F32)
    din("c3W", (64, 2), BF16)
    din("c3B", (2, 1), F32)

    out_dram = nc.dram_tensor("out", [2, 1], F32, kind="ExternalOutput").ap()

    with tile.TileContext(nc) as tc:
        _emit(tc, t, out_dram)

    nc.compile()
    return nc, set(t.keys())


def _emit(tc, t, out_dram):
    from contextlib import ExitStack
    nc = tc.nc
    ctx = ExitStack()

    # ---------------- pools ----------------
    constp = ctx.enter_context(tc.tile_pool(name="constp", bufs=1))
    wpool = ctx.enter_context(tc.tile_pool(name="wpool", bufs=2))
    actp = ctx.enter_context(tc.tile_pool(name="actp", bufs=1))
    esbp = ctx.enter_context(tc.tile_pool(name="esbp", bufs=6))
    lnp = ctx.enter_context(tc.tile_pool(name="lnp", bufs=1))
    sqp = ctx.enter_context(tc.tile_pool(name="sqp", bufs=3))
    recp = ctx.enter_context(tc.tile_pool(name="recp", bufs=2))
    w2p = ctx.enter_context(tc.tile_pool(name="w2p", bufs=3))
    bcp = ctx.enter_context(tc.tile_pool(name="bcp", bufs=1))
    dramp = ctx.enter_context(tc.tile_pool(name="dramp", bufs=1, space="DRAM"))
    psS = ctx.enter_context(tc.tile_pool(name="psS", bufs=3, space="PSUM"))
    psAV = ctx.enter_context(tc.tile_pool(name="psAV", bufs=2, space="PSUM"))
    psMM = ctx.enter_context(tc.tile_pool(name="psMM", bufs=2, space="PSUM"))
    psST = ctx.enter_context(tc.tile_pool(name="psST", bufs=1, space="PSUM"))

    def single(shape, dt, name, **kw):
        tl, free = tc.tile(shape, dt, name=name, **kw)
        ctx.callback(free)
        return tl

    # ---------------- hot-path loads first (stage-0 + embedding inputs) ----
    xloc_sb = constp.tile([P, SL], BF16, name="xloc_sb", tag="xloc_sb")
    nc.sync.dma_start(out=xloc_sb[:], in_=t["xloc"][:])
    embW_sb = constp.tile([P, D], BF16, name="embW_sb", tag="embW_sb")
    nc.sync.dma_start(out=embW_sb[:], in_=t["embW"][:])

    # ---------------- constants / small tensors (single packed DMA) -------
    NBC = 2 * NKC + NL * (8 * NKC + NFC)
    sb_bias = constp.tile([128, NBC], F32, name="sb_bias", tag="sb_bias")
    nc.sync.dma_start(out=sb_bias[:], in_=t["biases"][:])
    _bc = [0]

    def bias_col(n=NKC):
        c0 = _bc[0]
        _bc[0] += n
        return sb_bias[:, c0:c0 + n]

    sb_embWsumN = bias_col()
    sb_embB = bias_col()
    bias_sb = {}
    for l in range(NL):
        for v in ("bq", "bk", "bo", "b2", "ln1s", "ln1b", "ln2s", "ln2b"):
            bias_sb[f"{v}{l}"] = bias_col()
        bias_sb[f"b1{l}"] = bias_col(NFC)

    sb_bvp = constp.tile([1, NL * D], BF16, name="sb_bvp", tag="sb_bvp")
    nc.sync.dma_start(out=sb_bvp[:], in_=t["bvpack"][:])
    sb_bv = {l: sb_bvp[:, ds(l * D, D)] for l in range(NL)}

    sb_featC = constp.tile([OUT, 1], F32, name="sb_featC", tag="sb_featC")
    nc.sync.dma_start(out=sb_featC[:], in_=t["featC"][:])
    sb_hwsumN = constp.tile([1, OUT], BF16, name="sb_hwsumN", tag="sb_hwsumN")
    nc.sync.dma_start(out=sb_hwsumN[:], in_=t["hwsumN"][:])
    sb_c1B = constp.tile([128, 2], F32, name="sb_c1B", tag="sb_c1B")
    nc.sync.dma_start(out=sb_c1B[:], in_=t["c1B"][:])
    sb_c2B = constp.tile([64, 1], F32, name="sb_c2B", tag="sb_c2B")
    nc.sync.dma_start(out=sb_c2B[:], in_=t["c2B"][:])
    sb_c3B = constp.tile([2, 1], F32, name="sb_c3B", tag="sb_c3B")
    nc.sync.dma_start(out=sb_c3B[:], in_=t["c3B"][:])

    sb_c1W = constp.tile([OUT, 256], BF16, name="sb_c1W", tag="sb_c1W")
    nc.sync.dma_start(out=sb_c1W[:], in_=t["c1W"][:])
    sb_c2W = constp.tile([128, 2, 64], BF16, name="sb_c2W", tag="sb_c2W")
    nc.sync.dma_start(out=sb_c2W[:], in_=t["c2W"][:])
    sb_c3W = constp.tile([64, 2], BF16, name="sb_c3W", tag="sb_c3W")
    nc.sync.dma_start(out=sb_c3W[:], in_=t["c3W"][:])
    sb_headW = constp.tile([128, NKC, OUT], BF16, name="sb_headW", tag="sb_headW")
    nc.sync.dma_start(out=sb_headW[:], in_=t["headWs"][:])

    sb_mask = constp.tile([112, 6, SL], BF16, name="sb_mask", tag="sb_mask")
    nc.sync.dma_start(out=sb_mask[:], in_=t["maskM"][:])

    ones_bf = constp.tile([128, 1], BF16, name="ones_bf", tag="ones_bf")
    nc.vector.memset(ones_bf[:], 1.0)
    eps6_sb = constp.tile([C, 1], F32, name="eps6_sb", tag="eps6_sb")
    nc.vector.memset(eps6_sb[:], 1e-6)
    eps5_sb = constp.tile([1, 1], F32, name="eps5_sb", tag="eps5_sb")
    nc.vector.memset(eps5_sb[:], 1e-5)
    zero_sb = constp.tile([128, 1], F32, name="zero_sb", tag="zero_sb")
    nc.vector.memset(zero_sb[:], 0.0)

    # ---------------- stage 0: instance norm stats ----------------
    st6 = constp.tile([C, 6, 6], F32, name="st6", tag="st6")
    xfp = ctx.enter_context(tc.tile_pool(name="xfp", bufs=3))
    for i in range(6):
        xfc = xfp.tile([C, 512], F32, name="xfc", tag="xfc")
        nc.sync.dma_start(out=xfc[:], in_=t["xfull"][:, ts(i, 512)])
        nc.vector.bn_stats(out=st6[:, i, :], in_=xfc[:])
    mv = constp.tile([C, 2], F32, name="mv", tag="mv")
    nc.vector.bn_aggr(out=mv[:], in_=st6[:])
    std21 = constp.tile([C, 1], F32, name="std21", tag="std21")
    nc.scalar.activation(out=std21[:], in_=mv[:, 1:2], func=AF.Sqrt, bias=eps6_sb[:])
    stat2 = constp.tile([C, 2], F32, name="stat2", tag="stat2")
    nc.vector.reciprocal(out=stat2[:, 0:1], in_=std21[:])
    nc.vector.tensor_mul(stat2[:, 1:2], mv[:, 0:1], stat2[:, 0:1])

    stat_dram = dramp.tile([C, 2], F32, name="stat_dram", tag="stat_dram")
    nc.sync.dma_start(out=stat_dram[:], in_=stat2[:])
    # per-token [1, 336] vectors: token t -> channel c = t % 21
    rstd_tok = constp.tile([1, SL], F32, name="rstd_tok", tag="rstd_tok")
    nc.sync.dma_start(
        out=rstd_tok[:].rearrange("p (n c) -> p n c", c=C),
        in_=bass.AP(tensor=stat_dram[:].tensor, offset=stat_dram[:].offset,
                    ap=[[0, 16], [2, C]]))
    mrs_tok = constp.tile([1, SL], F32, name="mrs_tok", tag="mrs_tok")
    nc.sync.dma_start(
        out=mrs_tok[:].rearrange("p (n c) -> p n c", c=C),
        in_=bass.AP(tensor=stat_dram[:].tensor, offset=stat_dram[:].offset + 1,
                    ap=[[0, 16], [2, C]]))
    rt_b = constp.tile([128, SL], F32, name="rt_b", tag="rt_b")
    nc.gpsimd.partition_broadcast(out_ap=rt_b[:], in_ap=rstd_tok[:])
    mrs_b = constp.tile([128, SL], F32, name="mrs_b", tag="mrs_b")
    nc.gpsimd.partition_broadcast(out_ap=mrs_b[:], in_ap=mrs_tok[:])

    # ---------------- stage 1: embedding ----------------
    h_f32 = single([128, NKC, SL], F32, "h_f32")
    h_bf = single([128, NKC, SL], BF16, "h_bf")
    d1 = single([128, NKC, SL], F32, "d1")

    for c8 in range(NKC):
        pse = psMM.tile([128, SL], F32, name="pse", tag="mm")
        nc.tensor.matmul(pse[:], lhsT=embW_sb[:, ts(c8, 128)], rhs=xloc_sb[:],
                         start=True, stop=True)
        nc.vector.tensor_mul(d1[:, c8, :], pse[:], rt_b[:])
        nc.vector.scalar_tensor_tensor(
            out=d1[:, c8, :], in0=mrs_b[:], scalar=sb_embWsumN[:, c8:c8 + 1],
            in1=d1[:, c8, :], op0=AX.mult, op1=AX.add)
        nc.scalar.activation(out=h_f32[:, c8, :], in_=d1[:, c8, :],
                             func=AF.Identity, bias=sb_embB[:, c8:c8 + 1])
        nc.scalar.activation(out=h_bf[:, c8, :], in_=d1[:, c8, :],
                             func=AF.Identity, bias=sb_embB[:, c8:c8 + 1])

    # ---------------- helpers ----------------
    def load_w(src, n, tag="w"):
        """Load a pre-arranged [128, NKC, n] bf16 weight tile (ACT queue)."""
        w = wpool.tile([128, NKC, n], BF16, name="w_t", tag=tag)
        nc.scalar.dma_start(out=w[:], in_=src)
        return w

    def ln_stats(src, have_bf=False):
        """Returns (mean1, r1, r_b) for feature-major LN over src (f32)."""
        ps_sum = psST.tile([1, SL], F32, name="ps_sum", tag="st")
        ps_sq = psST.tile([1, SL], F32, name="ps_sq", tag="st")
        sqs = []
        for c8 in range(NKC):
            if not have_bf:
                nc.scalar.activation(out=h_bf[:, c8, :], in_=src[:, c8, :],
                                     func=AF.Identity, bias=zero_sb[:])
            sq_c = sqp.tile([128, SL], BF16, name="sq_c", tag="sq_c")
            nc.scalar.activation(out=sq_c[:], in_=src[:, c8, :], func=AF.Square)
            sqs.append(sq_c)
        for c8 in range(NKC):
            nc.tensor.matmul(ps_sum[:], lhsT=ones_bf[:],
                             rhs=h_bf[:, c8, :],
                             start=(c8 == 0), stop=(c8 == NKC - 1))
        for c8 in range(NKC):
            nc.tensor.matmul(ps_sq[:], lhsT=ones_bf[:],
                             rhs=sqs[c8][:],
                             start=(c8 == 0), stop=(c8 == NKC - 1))
        mean1 = lnp.tile([1, SL], F32, name="mean1", tag="mean1")
        nc.scalar.activation(out=mean1[:], in_=ps_sum[:], func=AF.Copy,
                             scale=1.0 / D)
        ms1 = lnp.tile([1, SL], F32, name="ms1", tag="ms1")
        nc.vector.tensor_mul(ms1[:], mean1[:], mean1[:])
        var1 = lnp.tile([1, SL], F32, name="var1", tag="var1")
        nc.vector.scalar_tensor_tensor(out=var1[:], in0=ps_sq[:], scalar=1.0 / D,
                                       in1=ms1[:], op0=AX.mult, op1=AX.subtract)
        std1 = lnp.tile([1, SL], F32, name="std1", tag="std1")
        nc.scalar.activation(out=std1[:], in_=var1[:], func=AF.Sqrt, bias=eps5_sb[:])
        r1 = lnp.tile([1, SL], F32, name="r1", tag="r1")
        nc.vector.reciprocal(out=r1[:], in_=std1[:])
        r_b = bcp.tile([128, SL], F32, name="r_b", tag="r_b")
        nc.gpsimd.partition_broadcast(out_ap=r_b[:], in_ap=r1[:])
        return mean1, r1, r_b

    def ln(s_sb, b_sb, write_bf, src):
        """Feature-major layernorm: reads src (f32), writes h_f32 (+h_bf)."""
        mean1, r1, r_b = ln_stats(src)
        mean_b = bcp.tile([128, SL], F32, name="mean_b", tag="mean_b")
        nc.gpsimd.partition_broadcast(out_ap=mean_b[:], in_ap=mean1[:])
        for c8 in range(NKC):
            nc.vector.tensor_sub(d1[:, c8, :], src[:, c8, :], mean_b[:])
            nc.vector.scalar_tensor_tensor(
                out=d1[:, c8, :], in0=d1[:, c8, :], scalar=s_sb[:, c8:c8 + 1],
                in1=r_b[:], op0=AX.mult, op1=AX.mult)
            nc.scalar.activation(out=h_f32[:, c8, :], in_=d1[:, c8, :],
                                 func=AF.Identity, bias=b_sb[:, c8:c8 + 1])
            if write_bf:
                nc.scalar.activation(out=h_bf[:, c8, :], in_=d1[:, c8, :],
                                     func=AF.Identity, bias=b_sb[:, c8:c8 + 1])

    # ---------------- transformer layers ----------------
    for l in range(NL):
        wk = load_w(t[f"Wk{l}"][:], D)
        wv = load_w(t[f"Wv{l}"][:], D)
        wq = load_w(t[f"Wq{l}"][:], D)
        KREG = 128 * NKC * SL
        VREG = SL * D

        # K feature-major projection -> bf16, bounce, AllGather (first)
        k_sb = actp.tile([128, NKC, SL], BF16, name="k_sb", tag="k_sb")
        bsb = bias_sb[f"bk{l}"]
        for oc in range(NKC):
            psp = psMM.tile([128, SL], F32, name="psp", tag="mm")
            for kc in range(NKC):
                nc.tensor.matmul(psp[:], lhsT=wk[:, kc, ts(oc, 128)],
                                 rhs=h_bf[:, kc, :],
                                 start=(kc == 0), stop=(kc == NKC - 1))
            nc.vector.tensor_scalar_add(k_sb[:, oc, :], psp[:],
                                        bsb[:, oc:oc + 1])
        KH = KREG // 2
        kbnc_in = [dramp.tile([KH], BF16, name=f"kbnc_in{l}_{i}",
                              tag=f"kbnc_in{l}_{i}") for i in range(2)]
        kbnc_out = [single([2 * KH], BF16, f"kbnc_out{l}_{i}", space="DRAM",
                           addr_space="Shared") for i in range(2)]
        for i in range(2):
            nc.sync.dma_start(
                out=kbnc_in[i][ds(0, KH)].rearrange("(kc p tk) -> p kc tk",
                                                    p=128, tk=SL),
                in_=k_sb[:, ds(i * 4, 4), :])
            nc.gpsimd.collective_compute(
                "AllGather", AX.bypass, replica_groups=REPLICA_GROUPS,
                ins=[kbnc_in[i][:]], outs=[kbnc_out[i][:]])

        # V token-major projection  [112, 3, 1024], bounce, AllGather
        bv_b = actp.tile([112, D], BF16, name="bv_b", tag="bv_b")
        nc.gpsimd.partition_broadcast(out_ap=bv_b[:], in_ap=sb_bv[l])
        v_sb = actp.tile([112, 3, D], BF16, name="v_sb", tag="v_sb")
        for tc3 in range(3):
            for nh in range(2):
                psv = psMM.tile([112, 512], F32, name="psv", tag="mm")
                for kc in range(NKC):
                    nc.tensor.matmul(psv[:], lhsT=h_bf[:, kc, ds(tc3 * 112, 112)],
                                     rhs=wv[:, kc, ts(nh, 512)],
                                     start=(kc == 0), stop=(kc == NKC - 1))
                nc.vector.tensor_add(
                    v_sb[:, tc3, ts(nh, 512)], psv[:], bv_b[:, ts(nh, 512)])
        VH = VREG // 2
        vbnc_in = [dramp.tile([VH], BF16, name=f"vbnc_in{l}_{i}",
                              tag=f"vbnc_in{l}_{i}") for i in range(2)]
        vbnc_out = [single([2 * VH], BF16, f"vbnc_out{l}_{i}", space="DRAM",
                           addr_space="Shared") for i in range(2)]
        for i in range(2):
            nc.sync.dma_start(
                out=vbnc_in[i][ds(0, VH)].rearrange("(t3 p he) -> p t3 he",
                                                    p=112, he=512),
                in_=v_sb[:, :, ds(i * 512, 512)])
            nc.gpsimd.collective_compute(
                "AllGather", AX.bypass, replica_groups=REPLICA_GROUPS,
                ins=[vbnc_in[i][:]], outs=[vbnc_out[i][:]])

        # Q feature-major projection (overlaps the AllGathers)
        q_sb = actp.tile([128, NKC, SL], BF16, name="q_sb", tag="q_sb")
        bsb = bias_sb[f"bq{l}"]
        for oc in range(NKC):
            psp = psMM.tile([128, SL], F32, name="psp", tag="mm")
            for kc in range(NKC):
                nc.tensor.matmul(psp[:], lhsT=wq[:, kc, ts(oc, 128)],
                                 rhs=h_bf[:, kc, :],
                                 start=(kc == 0), stop=(kc == NKC - 1))
            nc.vector.tensor_scalar_add(q_sb[:, oc, :], psp[:],
                                        bsb[:, oc:oc + 1])

        k_full = actp.tile([128, NKC, S], BF16, name="k_full", tag="k_full")
        for i in range(2):
            for r in range(2):
                nc.sync.dma_start(
                    out=k_full[:, ds(i * 4, 4), ds(r * SL, SL)],
                    in_=kbnc_out[i][ds(r * KH, KH)].rearrange(
                        "(kc p tk) -> p kc tk", p=128, tk=SL))
        v_full = actp.tile([112, 6, H, HD + 1], BF16, name="v_full", tag="v_full")
        nc.vector.memset(v_full[:, :, :, HD:HD + 1], 1.0)
        for i in range(2):
            for c6 in range(6):
                off = (c6 // 3) * VH + (c6 % 3) * 112 * 512
                nc.gpsimd.dma_start(
                    out=v_full[:, c6, ds(i * 8, 8), 0:HD],
                    in_=vbnc_out[i][ds(off, 112 * 512)].rearrange(
                        "(p hh e) -> p hh e", p=112, e=HD))

        # attention
        att_sb = actp.tile([128, NKC, SL], BF16, name="att_sb", tag="att_sb")
        stage_odd = actp.tile([64, NKC, SL], BF16, name="stage_odd", tag="stage_odd")
        for hh in range(H):
            hb2 = 64 * (hh % 2)
            hc = hh // 2
            esbs = []
            for cc in range(6):
                tqs = SUF0[cc]
                suf = SL - tqs
                pss = psS.tile([112, SL], F32, name="pss", tag="s")
                nc.tensor.matmul(pss[:, 0:suf],
                                 lhsT=k_full[ds(hb2, 64), hc, ts(cc, 112)],
                                 rhs=q_sb[ds(hb2, 64), hc, ds(tqs, suf)],
                                 start=True, stop=True)
                esb = esbp.tile([112, SL], BF16, name="esb", tag="esb")
                nc.scalar.activation(out=esb[:, 0:suf], in_=pss[:, 0:suf],
                                     func=AF.Exp)
                nc.vector.tensor_mul(esb[:, 0:suf], esb[:, 0:suf],
                                     sb_mask[:, cc, ds(tqs, suf)])
                esbs.append(esb)
            psa = psAV.tile([HD + 1, SL], F32, name="psa", tag="av")
            for cc in range(6):
                tqs = SUF0[cc]
                suf = SL - tqs
                nc.tensor.matmul(psa[:, ds(tqs, suf)],
                                 lhsT=v_full[:, cc, hh, :],
                                 rhs=esbs[cc][:, 0:suf],
                                 start=(cc == 0), stop=(cc == 5))
            rec = recp.tile([1, SL], F32, name="rec", tag="rec")
            nc.vector.reciprocal(out=rec[:], in_=psa[ds(HD, 1), :])
            rb = recp.tile([64, SL], F32, name="rb", tag="rb")
            nc.gpsimd.partition_broadcast(out_ap=rb[:], in_ap=rec[:])
            dst = att_sb[0:64, hc, :] if hh % 2 == 0 else stage_odd[:, hc, :]
            nc.vector.tensor_mul(dst, psa[0:HD, :], rb[:])
        nc.sync.dma_start(out=att_sb[ds(64, 64), :, :], in_=stage_odd[:])

        # out-proj + residual
        wo = load_w(t[f"Wo{l}"][:], D)
        bo_sb = bias_sb[f"bo{l}"]
        for oc in range(NKC):
            pso = psMM.tile([128, SL], F32, name="pso", tag="mm")
            for kc in range(NKC):
                nc.tensor.matmul(pso[:], lhsT=wo[:, kc, ts(oc, 128)],
                                 rhs=att_sb[:, kc, :],
                                 start=(kc == 0), stop=(kc == NKC - 1))
            nc.vector.scalar_tensor_tensor(
                out=d1[:, oc, :], in0=pso[:], scalar=bo_sb[:, oc:oc + 1],
                in1=h_f32[:, oc, :], op0=AX.add, op1=AX.add)

        ln(bias_sb[f"ln1s{l}"], bias_sb[f"ln1b{l}"], write_bf=True, src=d1)

        # FFN
        w1q = [load_w(t[f"W1{l}"][j], 1024) for j in range(4)]
        g_sb = actp.tile([128, NFC, SL], BF16, name="g_sb", tag="g_sb")
        b1_sb = bias_sb[f"b1{l}"]
        for fc in range(NFC):
            w1 = w1q[fc // 8]
            psf = psMM.tile([128, SL], F32, name="psf", tag="mm")
            for kc in range(NKC):
                nc.tensor.matmul(psf[:], lhsT=w1[:, kc, ts(fc % 8, 128)],
                                 rhs=h_bf[:, kc, :],
                                 start=(kc == 0), stop=(kc == NKC - 1))
            nc.scalar.activation(out=g_sb[:, fc, :], in_=psf[:], func=AF.Gelu,
                                 bias=b1_sb[:, fc:fc + 1])
        b2_sb = bias_sb[f"b2{l}"]
        for oc in range(NKC):
            w2 = w2p.tile([128, NFC, 128], BF16, name="w2oc", tag="w2oc")
            nc.scalar.dma_start(out=w2[:], in_=t[f"W2{l}"][oc])
            psy = psMM.tile([128, SL], F32, name="psy", tag="mm")
            for kc in range(NFC):
                nc.tensor.matmul(psy[:], lhsT=w2[:, kc, ts(0, 128)],
                                 rhs=g_sb[:, kc, :],
                                 start=(kc == 0), stop=(kc == NFC - 1))
            nc.vector.scalar_tensor_tensor(
                out=d1[:, oc, :], in0=psy[:], scalar=b2_sb[:, oc:oc + 1],
                in1=h_f32[:, oc, :], op0=AX.add, op1=AX.add)

        ln(bias_sb[f"ln2s{l}"], bias_sb[f"ln2b{l}"], write_bf=True, src=d1)

    # ---------------- final: fused LN_f + pooling, head, MLP ----------------
    # pooled[d] = (1/672) * s_d * (sum_t h[d,t]*r_t - sum_t m_t*r_t) + b_d
    mean1, r1, r_b = ln_stats(h_f32, have_bf=True)
    mr1 = lnp.tile([1, SL], F32, name="mr1", tag="mr1")
    nc.vector.tensor_mul(mr1[:], mean1[:], r1[:])
    csc = lnp.tile([1, 1], F32, name="csc", tag="csc")
    nc.vector.reduce_sum(out=csc[:], in_=mr1[:], axis=XL.X)
    hsum = constp.tile([128, NKC], F32, name="hsum", tag="hsum")
    for c8 in range(NKC):
        nc.vector.scalar_tensor_tensor(
            out=d1[:, c8, :], in0=h_f32[:, c8, :], scalar=1.0, in1=r_b[:],
            op0=AX.mult, op1=AX.mult, accum_out=hsum[:, c8:c8 + 1])
    FD = D + 8  # 32B-aligned block: 1024 sums + csc + pad
    fin_in = dramp.tile([FD], F32, name="fin_in", tag="fin_in")
    nc.sync.dma_start(out=fin_in[ds(0, D)].rearrange("(kc p) -> p kc", p=128),
                      in_=hsum[:])
    nc.sync.dma_start(out=fin_in[ds(D, 1)], in_=csc[:])
    fin_out = single([2 * FD], F32, "fin_out", space="DRAM",
                     addr_space="Shared")
    nc.gpsimd.collective_compute(
        "AllGather", AX.bypass, replica_groups=REPLICA_GROUPS,
        ins=[fin_in[:]], outs=[fin_out[:]])
    ffx = constp.tile([128, NKC, 2], F32, name="ffx", tag="ffx")
    for r in range(2):
        nc.sync.dma_start(
            out=ffx[:, :, r],
            in_=fin_out[ds(r * FD, D)].rearrange("(kc p) -> p kc", p=128))
    csc2 = constp.tile([1, 2], F32, name="csc2", tag="csc2")
    for r in range(2):
        nc.sync.dma_start(out=csc2[:, r:r + 1], in_=fin_out[ds(r * FD + D, 1)])
    cst_bf = constp.tile([1, 1], BF16, name="cst_bf", tag="cst_bf")
    nc.vector.tensor_add(cst_bf[:], csc2[:, 0:1], csc2[:, 1:2])
    hbar_bf = constp.tile([128, NKC], BF16, name="hbar_bf", tag="hbar_bf")
    nc.vector.tensor_add(hbar_bf[:], ffx[:, :, 0], ffx[:, :, 1])

    psh = psMM.tile([OUT, 1], F32, name="psh", tag="mm")
    for kc in range(NKC):
        nc.tensor.matmul(psh[:], lhsT=sb_headW[:, kc, :],
                         rhs=hbar_bf[:, kc:kc + 1],
                         start=(kc == 0), stop=False)
    nc.tensor.matmul(psh[:], lhsT=sb_hwsumN[:], rhs=cst_bf[:],
                     start=False, stop=True)
    feat_bf = constp.tile([OUT, 1], BF16, name="feat_bf", tag="feat_bf")
    nc.scalar.activation(out=feat_bf[:], in_=psh[:], func=AF.Identity,
                         bias=sb_featC[:])

    z1_bf = constp.tile([128, 2], BF16, name="z1_bf", tag="z1_bf")
    for i2 in range(2):
        psc = psMM.tile([128, 1], F32, name="psc", tag="mm")
        nc.tensor.matmul(psc[:], lhsT=sb_c1W[:, ts(i2, 128)], rhs=feat_bf[:],
                         start=True, stop=True)
        nc.scalar.activation(out=z1_bf[:, i2:i2 + 1], in_=psc[:], func=AF.Relu,
                             bias=sb_c1B[:, i2:i2 + 1])
    psc2 = psMM.tile([64, 1], F32, name="psc2", tag="mm")
    for kc in range(2):
        nc.tensor.matmul(psc2[:], lhsT=sb_c2W[:, kc, :], rhs=z1_bf[:, kc:kc + 1],
                         start=(kc == 0), stop=(kc == 1))
    z2_bf = constp.tile([64, 1], BF16, name="z2_bf", tag="z2_bf")
    nc.scalar.activation(out=z2_bf[:], in_=psc2[:], func=AF.Relu, bias=sb_c2B[:])
    psc3 = psMM.tile([2, 1], F32, name="psc3", tag="mm")
    nc.tensor.matmul(psc3[:], lhsT=sb_c3W[:], rhs=z2_bf[:], start=True, stop=True)
    out_sb = constp.tile([2, 1], F32, name="out_sb", tag="out_sb")
    nc.scalar.activation(out=out_sb[:], in_=psc3[:], func=AF.Identity,
                         bias=sb_c3B[:])
    nc.sync.dma_start(out=out_dram[:], in_=out_sb[:])
    ctx.close()


# ----------------------------------------------------------------------------
# host side
# ----------------------------------------------------------------------------

def _bf16(x):
    return np.ascontiguousarray(np.asarray(x, dtype=np.float32)).astype(
        ml_dtypes.bfloat16)


def _f32(x):
    return np.ascontiguousarray(np.asarray(x, dtype=np.float32))


def _wtile(a):
    # [D_in, N] -> [128, D_in//128, N] (p, kc, n)
    a = np.asarray(a, np.float32)
    din, n = a.shape
    return _bf16(a.reshape(din // 128, 128, n).transpose(1, 0, 2))


def _btile(a, p=128):
    # [dim] -> [p, dim//p]
    a = np.asarray(a, np.float32)
    return _f32(a.reshape(-1, p).T)


def _host_weights(inp):
    w = {}
    w["embW"] = _bf16(inp["emb_W"])
    bias_cols = [_btile(-np.asarray(inp["emb_W"], np.float32).sum(0)),
                 _btile(inp["emb_b"])]
    for l in range(NL):
        w[f"Wq{l}"] = _wtile(np.asarray(inp["Wq"][l], np.float32) * 0.125)
        w[f"Wk{l}"] = _wtile(inp["Wk"][l])
        w[f"Wv{l}"] = _wtile(inp["Wv"][l])
        w[f"Wo{l}"] = _wtile(inp["Wo"][l])
        w1 = np.asarray(inp["W1"][l], np.float32)
        w[f"W1{l}"] = _bf16(w1.reshape(NKC, 128, 4, 1024).transpose(2, 1, 0, 3))
        w2 = np.asarray(inp["W2"][l], np.float32)
        w[f"W2{l}"] = _bf16(w2.reshape(NFC, 128, NKC, 128).transpose(2, 1, 0, 3))
        bias_cols += [
            _btile(np.asarray(inp["bq"][l], np.float32) * 0.125),
            _btile(inp["bk"][l]),
            _btile(inp["bo"][l]),
            _btile(inp["b2"][l]),
            _btile(inp["ln1_s"][l]),
            _btile(inp["ln1_b"][l]),
            _btile(inp["ln2_s"][l]),
            _btile(inp["ln2_b"][l]),
            _btile(inp["b1"][l]),
        ]
    w["biases"] = _f32(np.concatenate(bias_cols, axis=1))
    w["bvpack"] = _bf16(np.concatenate(
        [np.asarray(inp["bv"][l], np.float32) for l in range(NL)])[None, :])
    lnfs = np.asarray(inp["lnf_s"], np.float32)
    lnfb = np.asarray(inp["lnf_b"], np.float32)
    hW = np.asarray(inp["head_W"], np.float32)
    hws = hW * lnfs[:, None] / S                       # [1024, 96]
    w["headWs"] = _wtile(hws)
    w["hwsumN"] = _bf16(-hws.sum(0)[None, :] * S / S)  # [1, 96]; times c/672 via S-scaled hws
    w["featC"] = _f32((hW.T @ lnfb + np.asarray(inp["head_b"], np.float32))[:, None])
    w["c1W"] = _bf16(inp["c1_W"])
    w["c1B"] = _btile(inp["c1_b"])
    w["c2W"] = _wtile(inp["c2_W"])
    w["c2B"] = _f32(np.asarray(inp["c2_b"], np.float32)[:, None])
    w["c3W"] = _bf16(inp["c3_W"])
    w["c3B"] = _f32(np.asarray(inp["c3_b"], np.float32)[:, None])
    return w


def kernel(**inputs):
    global _BUILT, LAST_RESULT
    if _BUILT is None:
        _BUILT = _build()
    nc, names = _BUILT

    w = _host_weights(inputs)
    x = np.asarray(inputs["x"], np.float32)  # [4, 3072, 21]

    # precomputed key-window map (global gathered order) and masks per parity
    wk = np.concatenate([np.repeat(np.arange(16) * 2, C),
                         np.repeat(np.arange(16) * 2 + 1, C)])  # [672]
    in_maps = []
    for core in range(8):
        b, parity = core // 2, core % 2
        wins = np.arange(16) * 2 + parity
        xb = x[b]  # [3072, 21]
        xl = np.empty((P, SL), np.float32)
        for i, wn in enumerate(wins):
            xl[:, i * C:(i + 1) * C] = xb[wn * P:(wn + 1) * P, :]
        wq = np.repeat(wins, C)
        mask = (wk[:, None] <= wq[None, :]).astype(np.float32)  # [672, 336]
        mask3 = mask.reshape(6, 112, SL).transpose(1, 0, 2)     # [112, 6, 336]
        m = dict(w)
        m["xfull"] = _f32(xb.T)
        m["xloc"] = _bf16(xl)
        m["maskM"] = _bf16(mask3)
        in_maps.append(m)

    res = run_bass_kernel_spmd(nc, in_maps, core_ids=list(range(8)))
    LAST_RESULT = res
    logits = np.stack(
        [res.results[2 * b]["out"].reshape(2).astype(np.float32) for b in range(B)])
    return logits



# revision 15
# speedup vs baseline: 1.1366x; 1.1366x over previous
"""Trainium2 Bass kernel for nn_Model_29592324670139 (dense transformer).

Sharding: 8 cores = 4 pairs. Pair b handles batch item b; within a pair the
672-token sequence (21 vars x 32 windows, window-major order) is split by
window parity (rank0 = even windows, rank1 = odd windows), 336 tokens each.
Per layer, each core projects Q/K/V for its tokens; K/V are exchanged within
the pair via a single bf16 AllReduce (partner = sum - own, which keeps the
program rank-symmetric); attention/FFN/LN run on local tokens. The final
pooled feature sum is AllGathered, and head+MLP run redundantly per pair.

Attention is two-pass: scores+exp against LOCAL keys for all 16 heads run
while the K/V exchange is in flight; remote-key scores, AV and the
normalization run in pass B. Key chunks are indexed by processing slot
(0-2 local, 3-5 remote); the block-causal mask comes in per-slot via a
6-row matmul accumulated into the score PSUM (one-hot key-window rows
dotted with -30 * [win > w_q] query rows), so no element-wise mask
multiply is needed.

Activations are feature-major ([d, token] with d on partitions). Matmuls in
bf16 with fp32 PSUM accumulation; softmax/LN statistics in fp32.

Self-contained: hardcodes all shapes; only needs numpy/ml_dtypes/concourse.
"""

import numpy as np
import ml_dtypes

import concourse.bass as bass
import concourse.tile as tile
from concourse import bacc, mybir
from concourse.bass import ts, ds
from concourse.bass_utils import run_bass_kernel_spmd

F32 = mybir.dt.float32
BF16 = mybir.dt.bfloat16
AX = mybir.AluOpType
AF = mybir.ActivationFunctionType
XL = mybir.AxisListType

B, L, C = 4, 3072, 21
P, OUT, D, H, NL, DFF = 96, 96, 1024, 16, 2, 4096
NW = 32          # windows
SL = 336         # local tokens per core
S = 672          # full sequence
HD = 64          # head dim
NKC = D // 128   # 8 k-chunks of d_model
NFC = DFF // 128  # 32 chunks of d_ff

REPLICA_GROUPS = [[0, 1], [2, 3], [4, 5], [6, 7]]

# query-suffix starts per key slot (block-causal skip); slot-order invariant
SUF0 = [0, 105, 210, 0, 105, 210]
# column offsets for the packed local-exp store (suffix widths 336/231/126)
EOFF = [0, 336, 567]

KREG = 128 * NKC * SL    # k elements exchanged
VREG = SL * D            # v elements exchanged

_BUILT = None  # cached (nc, input_names)

DEBUG_DUMP = False  # add dbg_in/dbg_out ExternalOutputs for exchange debug

LAST_RESULT = None  # stash of the last BassKernelResults (for test harness)


# ----------------------------------------------------------------------------
# device program
# ----------------------------------------------------------------------------

def _build():
    nc = bacc.Bacc("TRN2", target_bir_lowering=False, debug=False,
                   enable_asserts=False, num_devices=8)

    t = {}

    def din(name, shape, dt):
        t[name] = nc.dram_tensor(name, list(shape), dt, kind="ExternalInput").ap()

    # per-core data
    din("xfull", (C, L), F32)
    din("xloc", (P, SL), BF16)
    din("kextM", (6, 6, 112), BF16)
    din("qextM", (6, 6, SL), BF16)
    # embedding
    din("embW", (P, D), BF16)
    din("biases", (128, 2 * NKC + NL * (10 * NKC + NFC)), F32)  # packed vectors
    din("bvpack", (1, NL * D), BF16)
    for l in range(NL):
        for w in ("Wq", "Wk", "Wo", "Wv"):
            din(f"{w}{l}", (128, NKC, D), BF16)      # [p, kc, n] pre-arranged
        din(f"W1{l}", (4, 128, NKC, 1024), BF16)     # quarters [j, p, kc, n]
        din(f"W2{l}", (NKC, 128, NFC, 128), BF16)    # per-oc [oc, p, kc, c]
    din("headWs", (128, NKC, OUT), BF16)
    din("hwsumN", (1, OUT), BF16)
    din("featC", (OUT, 1), F32)
    din("c1W", (OUT, 256), BF16)
    din("c1B", (128, 2), F32)
    din("c2W", (128, 2, 64), BF16)
    din("c2B", (64, 1), F32)
    din("c3W", (64, 2), BF16)
    din("c3B", (2, 1), F32)

    out_dram = nc.dram_tensor("out", [2, 1], F32, kind="ExternalOutput").ap()
    if DEBUG_DUMP:
        t["dbg_in"] = nc.dram_tensor(
            "dbg_in", [KREG + VREG], BF16, kind="ExternalOutput").ap()
        t["dbg_out"] = nc.dram_tensor(
            "dbg_out", [KREG + VREG], BF16, kind="ExternalOutput").ap()
        t["dbg_q"] = nc.dram_tensor(
            "dbg_q", [128, NKC, SL], BF16, kind="ExternalOutput").ap()
        t["dbg_krem"] = nc.dram_tensor(
            "dbg_krem", [128, NKC, SL], BF16, kind="ExternalOutput").ap()
        t["dbg_vfull"] = nc.dram_tensor(
            "dbg_vfull", [112, 6, H, HD + 1], BF16, kind="ExternalOutput").ap()
        t["dbg_esb"] = nc.dram_tensor(
            "dbg_esb", [112, H, 693], BF16, kind="ExternalOutput").ap()
        t["dbg_att"] = nc.dram_tensor(
            "dbg_att", [128, NKC, SL], BF16, kind="ExternalOutput").ap()
        t["dbg_hln1"] = nc.dram_tensor(
            "dbg_hln1", [128, NKC, SL], F32, kind="ExternalOutput").ap()
        t["dbg_den"] = nc.dram_tensor(
            "dbg_den", [H, SL], F32, kind="ExternalOutput").ap()
        t["dbg_rec"] = nc.dram_tensor(
            "dbg_rec", [H, SL], F32, kind="ExternalOutput").ap()

    with tile.TileContext(nc) as tc:
        _emit(tc, t, out_dram)

    nc.compile()
    return nc, set(t.keys())


def _emit(tc, t, out_dram):
    from contextlib import ExitStack
    nc = tc.nc
    ctx = ExitStack()

    # ---------------- pools ----------------
    constp = ctx.enter_context(tc.tile_pool(name="constp", bufs=1))
    wpool = ctx.enter_context(tc.tile_pool(name="wpool", bufs=2))
    actp = ctx.enter_context(tc.tile_pool(name="actp", bufs=1))
    esbp = ctx.enter_context(tc.tile_pool(name="esbp", bufs=6))
    lnp = ctx.enter_context(tc.tile_pool(name="lnp", bufs=1))
    sqp = ctx.enter_context(tc.tile_pool(name="sqp", bufs=3))
    recp = ctx.enter_context(tc.tile_pool(name="recp", bufs=2))
    w2p = ctx.enter_context(tc.tile_pool(name="w2p", bufs=2 if not DEBUG_DUMP else 1))
    bcp = ctx.enter_context(tc.tile_pool(name="bcp", bufs=1))
    ap2 = ctx.enter_context(tc.tile_pool(name="ap2", bufs=3))
    dramp = ctx.enter_context(tc.tile_pool(name="dramp", bufs=1, space="DRAM"))
    psS = ctx.enter_context(tc.tile_pool(name="psS", bufs=3, space="PSUM"))
    psAV = ctx.enter_context(tc.tile_pool(name="psAV", bufs=2, space="PSUM"))
    psMM = ctx.enter_context(tc.tile_pool(name="psMM", bufs=2, space="PSUM"))
    psST = ctx.enter_context(tc.tile_pool(name="psST", bufs=1, space="PSUM"))

    def single(shape, dt, name, **kw):
        tl, free = tc.tile(shape, dt, name=name, **kw)
        ctx.callback(free)
        return tl

    # ---------------- hot-path loads first ------------------------------
    # stage-0 statistics input (longest dependency chain) goes first
    st6 = constp.tile([C, 6, 6], F32, name="st6", tag="st6")
    xfp = ctx.enter_context(tc.tile_pool(name="xfp", bufs=2))
    xfcs = []
    for i in range(6):
        xfc = xfp.tile([C, 512], F32, name="xfc", tag="xfc")
        nc.sync.dma_start(out=xfc[:], in_=t["xfull"][:, ts(i, 512)])
        xfcs.append(xfc)
    xloc_sb = constp.tile([P, SL], BF16, name="xloc_sb", tag="xloc_sb")
    nc.sync.dma_start(out=xloc_sb[:], in_=t["xloc"][:])
    embW_sb = constp.tile([P, D], BF16, name="embW_sb", tag="embW_sb")
    nc.sync.dma_start(out=embW_sb[:], in_=t["embW"][:])

    # layer-0 K/V weights prefetched before everything else on the ACT queue
    def load_w(src, n, tag="w"):
        """Load a pre-arranged [128, NKC, n] bf16 weight tile (ACT queue)."""
        w = wpool.tile([128, NKC, n], BF16, name="w_t", tag=tag)
        nc.scalar.dma_start(out=w[:], in_=src)
        return w

    wk = load_w(t["Wk0"][:], D)
    wv = load_w(t["Wv0"][:], D)

    # ---------------- constants / small tensors -------------------------
    NBC = 2 * NKC + NL * (10 * NKC + NFC)
    sb_bias = constp.tile([128, NBC], F32, name="sb_bias", tag="sb_bias")
    nc.sync.dma_start(out=sb_bias[:], in_=t["biases"][:])
    _bc = [0]

    def bias_col(n=NKC):
        c0 = _bc[0]
        _bc[0] += n
        return sb_bias[:, c0:c0 + n]

    sb_embWsumN = bias_col()
    sb_embB = bias_col()
    bias_sb = {}
    for l in range(NL):
        for v in ("bq", "bk", "bo", "b2", "ln1s", "ln1sneg", "ln1b",
                  "ln2s", "ln2sneg", "ln2b"):
            bias_sb[f"{v}{l}"] = bias_col()
        bias_sb[f"b1{l}"] = bias_col(NFC)

    sb_bvp = constp.tile([1, NL * D], BF16, name="sb_bvp", tag="sb_bvp")
    nc.sync.dma_start(out=sb_bvp[:], in_=t["bvpack"][:])
    sb_bv = {l: sb_bvp[:, ds(l * D, D)] for l in range(NL)}

    sb_kext = constp.tile([6, 6, 112], BF16, name="sb_kext", tag="sb_kext")
    nc.sync.dma_start(out=sb_kext[:], in_=t["kextM"][:])
    sb_qext = constp.tile([6, 6, SL], BF16, name="sb_qext", tag="sb_qext")
    nc.sync.dma_start(out=sb_qext[:], in_=t["qextM"][:])

    ones_bf = constp.tile([128, 1], BF16, name="ones_bf", tag="ones_bf")
    nc.vector.memset(ones_bf[:], 1.0)
    eps6_sb = constp.tile([C, 1], F32, name="eps6_sb", tag="eps6_sb")
    nc.vector.memset(eps6_sb[:], 1e-6)
    eps5_sb = constp.tile([1, 1], F32, name="eps5_sb", tag="eps5_sb")
    nc.vector.memset(eps5_sb[:], 1e-5)

    # ---------------- stage 0: instance norm stats ----------------
    for i in range(6):
        nc.vector.bn_stats(out=st6[:, i, :], in_=xfcs[i][:])
    mv = constp.tile([C, 2], F32, name="mv", tag="mv")
    nc.vector.bn_aggr(out=mv[:], in_=st6[:])
    std21 = constp.tile([C, 1], F32, name="std21", tag="std21")
    nc.scalar.activation(out=std21[:], in_=mv[:, 1:2], func=AF.Sqrt, bias=eps6_sb[:])
    stat2 = constp.tile([C, 2], F32, name="stat2", tag="stat2")
    nc.vector.reciprocal_approx_fast(out=stat2[:, 0:1], in_=std21[:])
    nc.vector.tensor_mul(stat2[:, 1:2], mv[:, 0:1], stat2[:, 0:1])

    stat_dram = dramp.tile([C, 2], F32, name="stat_dram", tag="stat_dram")
    nc.sync.dma_start(out=stat_dram[:], in_=stat2[:])
    # per-token [1, 336] vectors: token t -> channel c = t % 21
    rstd_tok = constp.tile([1, SL], F32, name="rstd_tok", tag="rstd_tok")
    nc.sync.dma_start(
        out=rstd_tok[:].rearrange("p (n c) -> p n c", c=C),
        in_=bass.AP(tensor=stat_dram[:].tensor, offset=stat_dram[:].offset,
                    ap=[[0, 16], [2, C]]))
    mrs_tok = constp.tile([1, SL], F32, name="mrs_tok", tag="mrs_tok")
    nc.sync.dma_start(
        out=mrs_tok[:].rearrange("p (n c) -> p n c", c=C),
        in_=bass.AP(tensor=stat_dram[:].tensor, offset=stat_dram[:].offset + 1,
                    ap=[[0, 16], [2, C]]))
    rt_b = constp.tile([128, SL], F32, name="rt_b", tag="rt_b")
    nc.gpsimd.partition_broadcast(out_ap=rt_b[:], in_ap=rstd_tok[:])
    mrs_b = constp.tile([128, SL], F32, name="mrs_b", tag="mrs_b")
    nc.gpsimd.partition_broadcast(out_ap=mrs_b[:], in_ap=mrs_tok[:])

    # ---------------- stage 1: embedding ----------------
    h_f32 = single([128, NKC, SL], F32, "h_f32")
    h_bf = single([128, NKC, SL], BF16, "h_bf")

    for c8 in range(NKC):
        pse = psMM.tile([128, SL], F32, name="pse", tag="mm")
        nc.tensor.matmul(pse[:], lhsT=embW_sb[:, ts(c8, 128)], rhs=xloc_sb[:],
                         start=True, stop=True)
        aa = ap2.tile([128, SL], F32, name="aa", tag="aa")
        nc.vector.tensor_mul(aa[:], pse[:], rt_b[:])
        nc.vector.scalar_tensor_tensor(
            out=aa[:], in0=mrs_b[:], scalar=sb_embWsumN[:, c8:c8 + 1],
            in1=aa[:], op0=AX.mult, op1=AX.add)
        nc.scalar.activation(out=h_f32[:, c8, :], in_=aa[:],
                             func=AF.Identity, bias=sb_embB[:, c8:c8 + 1])
        nc.scalar.activation(out=h_bf[:, c8, :], in_=aa[:],
                             func=AF.Identity, bias=sb_embB[:, c8:c8 + 1])

    # ---------------- helpers ----------------
    def ln_stats(src, have_bf=False):
        """Returns (mean1, r1, r_b) for feature-major LN over src (f32)."""
        ps_sum = psST.tile([1, SL], F32, name="ps_sum", tag="st")
        ps_sq = psST.tile([1, SL], F32, name="ps_sq", tag="st")
        sqs = []
        for c8 in range(NKC):
            if not have_bf:
                nc.scalar.activation(out=h_bf[:, c8, :], in_=src[:, c8, :],
                                     func=AF.Copy)
            sq_c = sqp.tile([128, SL], BF16, name="sq_c", tag="sq_c")
            nc.scalar.activation(out=sq_c[:], in_=src[:, c8, :], func=AF.Square)
            sqs.append(sq_c)
        for c8 in range(NKC):
            nc.tensor.matmul(ps_sum[:], lhsT=ones_bf[:],
                             rhs=h_bf[:, c8, :],
                             start=(c8 == 0), stop=(c8 == NKC - 1))
        for c8 in range(NKC):
            nc.tensor.matmul(ps_sq[:], lhsT=ones_bf[:],
                             rhs=sqs[c8][:],
                             start=(c8 == 0), stop=(c8 == NKC - 1))
        mean1 = lnp.tile([1, SL], F32, name="mean1", tag="mean1")
        nc.scalar.activation(out=mean1[:], in_=ps_sum[:], func=AF.Copy,
                             scale=1.0 / D)
        ms1 = lnp.tile([1, SL], F32, name="ms1", tag="ms1")
        nc.vector.tensor_mul(ms1[:], mean1[:], mean1[:])
        var1 = lnp.tile([1, SL], F32, name="var1", tag="var1")
        nc.vector.scalar_tensor_tensor(out=var1[:], in0=ps_sq[:], scalar=1.0 / D,
                                       in1=ms1[:], op0=AX.mult, op1=AX.subtract)
        std1 = lnp.tile([1, SL], F32, name="std1", tag="std1")
        nc.scalar.activation(out=std1[:], in_=var1[:], func=AF.Sqrt, bias=eps5_sb[:])
        r1 = lnp.tile([1, SL], F32, name="r1", tag="r1")
        nc.vector.reciprocal_approx_fast(out=r1[:], in_=std1[:])
        r_b = bcp.tile([128, SL], F32, name="r_b", tag="r_b")
        nc.gpsimd.partition_broadcast(out_ap=r_b[:], in_ap=r1[:])
        return mean1, r1, r_b

    def ln(s_sb, sneg_sb, b_sb, src):
        """Feature-major layernorm in place: src (f32) -> src + h_bf."""
        mean1, r1, r_b = ln_stats(src)
        mr1 = lnp.tile([1, SL], F32, name="mr1", tag="mr1")
        nc.vector.tensor_mul(mr1[:], mean1[:], r1[:])
        mrb = bcp.tile([128, SL], F32, name="mrb", tag="mrb")
        nc.gpsimd.partition_broadcast(out_ap=mrb[:], in_ap=mr1[:])
        for c8 in range(NKC):
            aa = ap2.tile([128, SL], F32, name="aa", tag="aa")
            # aa = (src * s) * r
            nc.vector.scalar_tensor_tensor(
                out=aa[:], in0=src[:, c8, :], scalar=s_sb[:, c8:c8 + 1],
                in1=r_b[:], op0=AX.mult, op1=AX.mult)
            # src = (mrb * -s) + aa   ==  (src - m) * r * s
            nc.vector.scalar_tensor_tensor(
                out=src[:, c8, :], in0=mrb[:], scalar=sneg_sb[:, c8:c8 + 1],
                in1=aa[:], op0=AX.mult, op1=AX.add)
            nc.scalar.activation(out=src[:, c8, :], in_=src[:, c8, :],
                                 func=AF.Identity, bias=b_sb[:, c8:c8 + 1])
            nc.scalar.activation(out=h_bf[:, c8, :], in_=src[:, c8, :],
                                 func=AF.Copy)

    # ---------------- transformer layers ----------------
    for l in range(NL):
        if l > 0:
            wk = load_w(t[f"Wk{l}"][:], D)
            wv = load_w(t[f"Wv{l}"][:], D)

        # K feature-major projection -> bf16 (bias via ACT)
        k_sb = actp.tile([128, NKC, SL], BF16, name="k_sb", tag="k_sb")
        bsb = bias_sb[f"bk{l}"]
        for oc in range(NKC):
            psp = psMM.tile([128, SL], F32, name="psp", tag="mm")
            for kc in range(NKC):
                nc.tensor.matmul(psp[:], lhsT=wk[:, kc, ts(oc, 128)],
                                 rhs=h_bf[:, kc, :],
                                 start=(kc == 0), stop=(kc == NKC - 1))
            nc.scalar.activation(out=k_sb[:, oc, :], in_=psp[:],
                                 func=AF.Identity, bias=bsb[:, oc:oc + 1])

        # V token-major projection [112, 3, 1024] (two PSUM banks per tchunk)
        bv_b = actp.tile([112, D], BF16, name="bv_b", tag="bv_b")
        nc.gpsimd.partition_broadcast(out_ap=bv_b[:], in_ap=sb_bv[l])
        v_sb = actp.tile([112, 3, D], BF16, name="v_sb", tag="v_sb")
        for tc3 in range(3):
            psv0 = psMM.tile([112, 512], F32, name="psv", tag="mm")
            psv1 = psMM.tile([112, 512], F32, name="psv", tag="mm")
            for kc in range(NKC):
                nc.tensor.matmul(psv0[:], lhsT=h_bf[:, kc, ds(tc3 * 112, 112)],
                                 rhs=wv[:, kc, ts(0, 512)],
                                 start=(kc == 0), stop=(kc == NKC - 1))
                nc.tensor.matmul(psv1[:], lhsT=h_bf[:, kc, ds(tc3 * 112, 112)],
                                 rhs=wv[:, kc, ts(1, 512)],
                                 start=(kc == 0), stop=(kc == NKC - 1))
            nc.vector.tensor_add(
                v_sb[:, tc3, ts(0, 512)], psv0[:], bv_b[:, ts(0, 512)])
            nc.vector.tensor_add(
                v_sb[:, tc3, ts(1, 512)], psv1[:], bv_b[:, ts(1, 512)])

        # ---- single combined K+V pair-exchange: AllReduce(sum), partner
        # recovered as sum - own (rank-symmetric program)
        kv_in = dramp.tile([KREG + VREG], BF16, name=f"kv_in{l}",
                           tag=f"kv_in{l}")
        kv_out = single([KREG + VREG], BF16, f"kv_out{l}", space="DRAM",
                        addr_space="Shared")
        nc.sync.dma_start(
            out=kv_in[ds(0, KREG)].rearrange("(kc p tk) -> p kc tk",
                                             p=128, tk=SL),
            in_=k_sb[:, :, :])
        nc.sync.dma_start(
            out=kv_in[ds(KREG, VREG)].rearrange("(t3 p he) -> p t3 he",
                                                p=112, he=D),
            in_=v_sb[:, :, :])
        nc.gpsimd.collective_compute(
            "AllReduce", AX.add, replica_groups=REPLICA_GROUPS,
            ins=[kv_in[:]], outs=[kv_out[:]])
        if DEBUG_DUMP and l == 0:
            nc.sync.dma_start(out=t["dbg_in"][:], in_=kv_in[:])
            nc.sync.dma_start(out=t["dbg_out"][:], in_=kv_out[:])

        # Q feature-major projection (overlaps the exchange)
        wq = load_w(t[f"Wq{l}"][:], D)
        q_sb = actp.tile([128, NKC, SL], BF16, name="q_sb", tag="q_sb")
        bsb = bias_sb[f"bq{l}"]
        for oc in range(NKC):
            psp = psMM.tile([128, SL], F32, name="psp", tag="mm")
            for kc in range(NKC):
                nc.tensor.matmul(psp[:], lhsT=wq[:, kc, ts(oc, 128)],
                                 rhs=h_bf[:, kc, :],
                                 start=(kc == 0), stop=(kc == NKC - 1))
            nc.scalar.activation(out=q_sb[:, oc, :], in_=psp[:],
                                 func=AF.Identity, bias=bsb[:, oc:oc + 1])

        # ---- attention pass A: local-key scores + exp (no exchange dep)
        esb_loc = actp.tile([112, H, 693], BF16, name="esb_loc", tag="esb_loc")
        for hh in range(H):
            hb2 = 64 * (hh % 2)
            hc = hh // 2
            for j in range(3):
                tqs = SUF0[j]
                suf = SL - tqs
                pss = psS.tile([112, SL], F32, name="pss", tag="s")
                nc.tensor.matmul(pss[:, 0:suf],
                                 lhsT=k_sb[ds(hb2, 64), hc, ts(j, 112)],
                                 rhs=q_sb[ds(hb2, 64), hc, ds(tqs, suf)],
                                 start=True, stop=False)
                nc.tensor.matmul(pss[:, 0:suf],
                                 lhsT=sb_kext[:, j, :],
                                 rhs=sb_qext[:, j, ds(tqs, suf)],
                                 start=False, stop=True)
                nc.scalar.activation(out=esb_loc[:, hh, ds(EOFF[j], suf)],
                                     in_=pss[:, 0:suf], func=AF.Exp)

        # ---- unpack exchange: remote K tile + slot-ordered V (with ones row)
        k_rem = actp.tile([128, NKC, SL], BF16, name="k_rem", tag="k_rem")
        nc.sync.dma_start(
            out=k_rem[:, :, :],
            in_=kv_out[ds(0, KREG)].rearrange("(kc p tk) -> p kc tk",
                                              p=128, tk=SL))
        nc.vector.scalar_tensor_tensor(
            out=k_rem[:, :, :], in0=k_rem[:, :, :], scalar=1.0,
            in1=k_sb[:, :, :], op0=AX.mult, op1=AX.subtract)

        v_full = actp.tile([112, 6, H, HD + 1], BF16, name="v_full", tag="v_full")
        nc.vector.memset(v_full[:, :, :, HD:HD + 1], 1.0)
        for t3 in range(3):
            nc.gpsimd.dma_start(
                out=v_full[:, t3, :, 0:HD],
                in_=v_sb[:, t3, :].rearrange("p (hh e) -> p hh e", e=HD))
            nc.gpsimd.dma_start(
                out=v_full[:, 3 + t3, :, 0:HD],
                in_=kv_out[ds(KREG + t3 * 112 * D, 112 * D)].rearrange(
                    "(p hh e) -> p hh e", p=112, e=HD))
        nc.vector.scalar_tensor_tensor(
            out=v_full[:, 3:6, :, 0:HD], in0=v_full[:, 3:6, :, 0:HD],
            scalar=1.0, in1=v_full[:, 0:3, :, 0:HD],
            op0=AX.mult, op1=AX.subtract)

        if DEBUG_DUMP and l == 0:
            nc.sync.dma_start(out=t["dbg_q"][:], in_=q_sb[:])
            nc.sync.dma_start(out=t["dbg_krem"][:], in_=k_rem[:])
            nc.sync.dma_start(out=t["dbg_vfull"][:], in_=v_full[:])
        # ---- attention pass B: remote scores + AV + normalize
        att_sb = actp.tile([128, NKC, SL], BF16, name="att_sb", tag="att_sb")
        stage_odd = actp.tile([64, NKC, SL], BF16, name="stage_odd",
                              tag="stage_odd")
        for hh in range(H):
            hb2 = 64 * (hh % 2)
            hc = hh // 2
            esbs = []
            for j in range(3):
                cc = 3 + j
                tqs = SUF0[cc]
                suf = SL - tqs
                pss = psS.tile([112, SL], F32, name="pss", tag="s")
                nc.tensor.matmul(pss[:, 0:suf],
                                 lhsT=k_rem[ds(hb2, 64), hc, ts(j, 112)],
                                 rhs=q_sb[ds(hb2, 64), hc, ds(tqs, suf)],
                                 start=True, stop=False)
                nc.tensor.matmul(pss[:, 0:suf],
                                 lhsT=sb_kext[:, cc, :],
                                 rhs=sb_qext[:, cc, ds(tqs, suf)],
                                 start=False, stop=True)
                esb = esbp.tile([112, SL], BF16, name="esb", tag="esb")
                nc.scalar.activation(out=esb[:, 0:suf], in_=pss[:, 0:suf],
                                     func=AF.Exp)
                esbs.append(esb)
            psa = psAV.tile([HD + 1, SL], F32, name="psa", tag="av")
            for j in range(3):
                tqs = SUF0[j]
                suf = SL - tqs
                nc.tensor.matmul(psa[:, ds(tqs, suf)],
                                 lhsT=v_full[:, j, hh, :],
                                 rhs=esb_loc[:, hh, ds(EOFF[j], suf)],
                                 start=(j == 0), stop=False)
            for j in range(3):
                tqs = SUF0[3 + j]
                suf = SL - tqs
                nc.tensor.matmul(psa[:, ds(tqs, suf)],
                                 lhsT=v_full[:, 3 + j, hh, :],
                                 rhs=esbs[j][:, 0:suf],
                                 start=False, stop=(j == 2))
            den = recp.tile([1, SL], F32, name="den", tag="den")
            nc.scalar.activation(out=den[:], in_=psa[ds(HD, 1), :],
                                 func=AF.Copy)
            rec = recp.tile([1, SL], F32, name="rec", tag="rec")
            nc.vector.reciprocal_approx_fast(out=rec[:], in_=den[:])
            rb = recp.tile([64, SL], F32, name="rb", tag="rb")
            nc.gpsimd.partition_broadcast(out_ap=rb[:], in_ap=rec[:])
            dst = att_sb[0:64, hc, :] if hh % 2 == 0 else stage_odd[:, hc, :]
            nc.vector.tensor_mul(dst, psa[0:HD, :], rb[:])
        nc.sync.dma_start(out=att_sb[ds(64, 64), :, :], in_=stage_odd[:])

        if DEBUG_DUMP and l == 0:
            nc.sync.dma_start(out=t["dbg_esb"][:], in_=esb_loc[:])
            nc.sync.dma_start(out=t["dbg_att"][:], in_=att_sb[:])
        # out-proj + residual (in place into h_f32)
        wo = load_w(t[f"Wo{l}"][:], D)
        bo_sb = bias_sb[f"bo{l}"]
        for oc in range(NKC):
            pso = psMM.tile([128, SL], F32, name="pso", tag="mm")
            for kc in range(NKC):
                nc.tensor.matmul(pso[:], lhsT=wo[:, kc, ts(oc, 128)],
                                 rhs=att_sb[:, kc, :],
                                 start=(kc == 0), stop=(kc == NKC - 1))
            nc.vector.scalar_tensor_tensor(
                out=h_f32[:, oc, :], in0=pso[:], scalar=bo_sb[:, oc:oc + 1],
                in1=h_f32[:, oc, :], op0=AX.add, op1=AX.add)

        ln(bias_sb[f"ln1s{l}"], bias_sb[f"ln1sneg{l}"], bias_sb[f"ln1b{l}"],
           src=h_f32)
        if DEBUG_DUMP and l == 0:
            nc.sync.dma_start(out=t["dbg_hln1"][:], in_=h_f32[:])

        # FFN
        w1q = [load_w(t[f"W1{l}"][j], 1024) for j in range(4)]
        g_sb = actp.tile([128, NFC, SL], BF16, name="g_sb", tag="g_sb")
        b1_sb = bias_sb[f"b1{l}"]
        for fc in range(NFC):
            w1 = w1q[fc // 8]
            psf = psMM.tile([128, SL], F32, name="psf", tag="mm")
            for kc in range(NKC):
                nc.tensor.matmul(psf[:], lhsT=w1[:, kc, ts(fc % 8, 128)],
                                 rhs=h_bf[:, kc, :],
                                 start=(kc == 0), stop=(kc == NKC - 1))
            nc.scalar.activation(out=g_sb[:, fc, :], in_=psf[:], func=AF.Gelu,
                                 bias=b1_sb[:, fc:fc + 1])
        b2_sb = bias_sb[f"b2{l}"]
        for oc in range(NKC):
            w2 = w2p.tile([128, NFC, 128], BF16, name="w2oc", tag="w2oc")
            nc.gpsimd.dma_start(out=w2[:], in_=t[f"W2{l}"][oc])
            psy = psMM.tile([128, SL], F32, name="psy", tag="mm")
            for kc in range(NFC):
                nc.tensor.matmul(psy[:], lhsT=w2[:, kc, ts(0, 128)],
                                 rhs=g_sb[:, kc, :],
                                 start=(kc == 0), stop=(kc == NFC - 1))
            nc.vector.scalar_tensor_tensor(
                out=h_f32[:, oc, :], in0=psy[:], scalar=b2_sb[:, oc:oc + 1],
                in1=h_f32[:, oc, :], op0=AX.add, op1=AX.add)

        ln(bias_sb[f"ln2s{l}"], bias_sb[f"ln2sneg{l}"], bias_sb[f"ln2b{l}"],
           src=h_f32)

    # ---------------- final: deferred const loads ----------------
    sb_featC = constp.tile([OUT, 1], F32, name="sb_featC", tag="sb_featC")
    nc.sync.dma_start(out=sb_featC[:], in_=t["featC"][:])
    sb_hwsumN = constp.tile([1, OUT], BF16, name="sb_hwsumN", tag="sb_hwsumN")
    nc.sync.dma_start(out=sb_hwsumN[:], in_=t["hwsumN"][:])
    sb_c1B = constp.tile([128, 2], F32, name="sb_c1B", tag="sb_c1B")
    nc.sync.dma_start(out=sb_c1B[:], in_=t["c1B"][:])
    sb_c2B = constp.tile([64, 1], F32, name="sb_c2B", tag="sb_c2B")
    nc.sync.dma_start(out=sb_c2B[:], in_=t["c2B"][:])
    sb_c3B = constp.tile([2, 1], F32, name="sb_c3B", tag="sb_c3B")
    nc.sync.dma_start(out=sb_c3B[:], in_=t["c3B"][:])
    sb_c1W = constp.tile([OUT, 256], BF16, name="sb_c1W", tag="sb_c1W")
    nc.sync.dma_start(out=sb_c1W[:], in_=t["c1W"][:])
    sb_c2W = constp.tile([128, 2, 64], BF16, name="sb_c2W", tag="sb_c2W")
    nc.sync.dma_start(out=sb_c2W[:], in_=t["c2W"][:])
    sb_c3W = constp.tile([64, 2], BF16, name="sb_c3W", tag="sb_c3W")
    nc.sync.dma_start(out=sb_c3W[:], in_=t["c3W"][:])
    sb_headW = constp.tile([128, NKC, OUT], BF16, name="sb_headW", tag="sb_headW")
    nc.sync.dma_start(out=sb_headW[:], in_=t["headWs"][:])

    # ---------------- final: fused LN_f + pooling, head, MLP ----------------
    # pooled[d] = (1/672) * s_d * (sum_t h[d,t]*r_t - sum_t m_t*r_t) + b_d
    mean1, r1, r_b = ln_stats(h_f32, have_bf=True)
    mr1 = lnp.tile([1, SL], F32, name="mr1", tag="mr1")
    nc.vector.tensor_mul(mr1[:], mean1[:], r1[:])
    csc = lnp.tile([1, 1], F32, name="csc", tag="csc")
    nc.vector.reduce_sum(out=csc[:], in_=mr1[:], axis=XL.X)
    hsum = constp.tile([128, NKC], F32, name="hsum", tag="hsum")
    for c8 in range(NKC):
        aa = ap2.tile([128, SL], F32, name="aa", tag="aa")
        nc.vector.scalar_tensor_tensor(
            out=aa[:], in0=h_f32[:, c8, :], scalar=1.0, in1=r_b[:],
            op0=AX.mult, op1=AX.mult, accum_out=hsum[:, c8:c8 + 1])
    FD = D + 8  # 32B-aligned block: 1024 sums + csc + pad
    fin_in = dramp.tile([FD], F32, name="fin_in", tag="fin_in")
    nc.sync.dma_start(out=fin_in[ds(0, D)].rearrange("(kc p) -> p kc", p=128),
                      in_=hsum[:])
    nc.sync.dma_start(out=fin_in[ds(D, 1)], in_=csc[:])
    fin_out = single([2 * FD], F32, "fin_out", space="DRAM",
                     addr_space="Shared")
    nc.gpsimd.collective_compute(
        "AllGather", AX.bypass, replica_groups=REPLICA_GROUPS,
        ins=[fin_in[:]], outs=[fin_out[:]])
    ffx = constp.tile([128, NKC, 2], F32, name="ffx", tag="ffx")
    for r in range(2):
        nc.sync.dma_start(
            out=ffx[:, :, r],
            in_=fin_out[ds(r * FD, D)].rearrange("(kc p) -> p kc", p=128))
    csc2 = constp.tile([1, 2], F32, name="csc2", tag="csc2")
    for r in range(2):
        nc.sync.dma_start(out=csc2[:, r:r + 1], in_=fin_out[ds(r * FD + D, 1)])
    cst_bf = constp.tile([1, 1], BF16, name="cst_bf", tag="cst_bf")
    nc.vector.tensor_add(cst_bf[:], csc2[:, 0:1], csc2[:, 1:2])
    hbar_bf = constp.tile([128, NKC], BF16, name="hbar_bf", tag="hbar_bf")
    nc.vector.tensor_add(hbar_bf[:], ffx[:, :, 0], ffx[:, :, 1])

    psh = psMM.tile([OUT, 1], F32, name="psh", tag="mm")
    for kc in range(NKC):
        nc.tensor.matmul(psh[:], lhsT=sb_headW[:, kc, :],
                         rhs=hbar_bf[:, kc:kc + 1],
                         start=(kc == 0), stop=False)
    nc.tensor.matmul(psh[:], lhsT=sb_hwsumN[:], rhs=cst_bf[:],
                     start=False, stop=True)
    feat_bf = constp.tile([OUT, 1], BF16, name="feat_bf", tag="feat_bf")
    nc.scalar.activation(out=feat_bf[:], in_=psh[:], func=AF.Identity,
                         bias=sb_featC[:])

    z1_bf = constp.tile([128, 2], BF16, name="z1_bf", tag="z1_bf")
    for i2 in range(2):
        psc = psMM.tile([128, 1], F32, name="psc", tag="mm")
        nc.tensor.matmul(psc[:], lhsT=sb_c1W[:, ts(i2, 128)], rhs=feat_bf[:],
                         start=True, stop=True)
        nc.scalar.activation(out=z1_bf[:, i2:i2 + 1], in_=psc[:], func=AF.Relu,
                             bias=sb_c1B[:, i2:i2 + 1])
    psc2 = psMM.tile([64, 1], F32, name="psc2", tag="mm")
    for kc in range(2):
        nc.tensor.matmul(psc2[:], lhsT=sb_c2W[:, kc, :], rhs=z1_bf[:, kc:kc + 1],
                         start=(kc == 0), stop=(kc == 1))
    z2_bf = constp.tile([64, 1], BF16, name="z2_bf", tag="z2_bf")
    nc.scalar.activation(out=z2_bf[:], in_=psc2[:], func=AF.Relu, bias=sb_c2B[:])
    psc3 = psMM.tile([2, 1], F32, name="psc3", tag="mm")
    nc.tensor.matmul(psc3[:], lhsT=sb_c3W[:], rhs=z2_bf[:], start=True, stop=True)
    out_sb = constp.tile([2, 1], F32, name="out_sb", tag="out_sb")
    nc.scalar.activation(out=out_sb[:], in_=psc3[:], func=AF.Identity,
                         bias=sb_c3B[:])
    nc.sync.dma_start(out=out_dram[:], in_=out_sb[:])
    ctx.close()


# ----------------------------------------------------------------------------
# host side
# ----------------------------------------------------------------------------

def _bf16(x):
    return np.ascontiguousarray(np.asarray(x, dtype=np.float32)).astype(
        ml_dtypes.bfloat16)


def _f32(x):
    return np.ascontiguousarray(np.asarray(x, dtype=np.float32))


def _wtile(a):
    # [D_in, N] -> [128, D_in//128, N] (p, kc, n)
    a = np.asarray(a, np.float32)
    din, n = a.shape
    return _bf16(a.reshape(din // 128, 128, n).transpose(1, 0, 2))


def _btile(a, p=128):
    # [dim] -> [p, dim//p]
    a = np.asarray(a, np.float32)
    return _f32(a.reshape(-1, p).T)


def _host_weights(inp):
    w = {}
    w["embW"] = _bf16(inp["emb_W"])
    bias_cols = [_btile(-np.asarray(inp["emb_W"], np.float32).sum(0)),
                 _btile(inp["emb_b"])]
    for l in range(NL):
        w[f"Wq{l}"] = _wtile(np.asarray(inp["Wq"][l], np.float32) * 0.125)
        w[f"Wk{l}"] = _wtile(inp["Wk"][l])
        w[f"Wv{l}"] = _wtile(inp["Wv"][l])
        w[f"Wo{l}"] = _wtile(inp["Wo"][l])
        w1 = np.asarray(inp["W1"][l], np.float32)
        w[f"W1{l}"] = _bf16(w1.reshape(NKC, 128, 4, 1024).transpose(2, 1, 0, 3))
        w2 = np.asarray(inp["W2"][l], np.float32)
        w[f"W2{l}"] = _bf16(w2.reshape(NFC, 128, NKC, 128).transpose(2, 1, 0, 3))
        ln1s = np.asarray(inp["ln1_s"][l], np.float32)
        ln2s = np.asarray(inp["ln2_s"][l], np.float32)
        bias_cols += [
            _btile(np.asarray(inp["bq"][l], np.float32) * 0.125),
            _btile(inp["bk"][l]),
            _btile(inp["bo"][l]),
            _btile(inp["b2"][l]),
            _btile(ln1s),
            _btile(-ln1s),
            _btile(inp["ln1_b"][l]),
            _btile(ln2s),
            _btile(-ln2s),
            _btile(inp["ln2_b"][l]),
            _btile(inp["b1"][l]),
        ]
    w["biases"] = _f32(np.concatenate(bias_cols, axis=1))
    w["bvpack"] = _bf16(np.concatenate(
        [np.asarray(inp["bv"][l], np.float32) for l in range(NL)])[None, :])
    lnfs = np.asarray(inp["lnf_s"], np.float32)
    lnfb = np.asarray(inp["lnf_b"], np.float32)
    hW = np.asarray(inp["head_W"], np.float32)
    hws = hW * lnfs[:, None] / S                       # [1024, 96]
    w["headWs"] = _wtile(hws)
    w["hwsumN"] = _bf16(-hws.sum(0)[None, :] * S / S)  # [1, 96]; times c/672 via S-scaled hws
    w["featC"] = _f32((hW.T @ lnfb + np.asarray(inp["head_b"], np.float32))[:, None])
    w["c1W"] = _bf16(inp["c1_W"])
    w["c1B"] = _btile(inp["c1_b"])
    w["c2W"] = _wtile(inp["c2_W"])
    w["c2B"] = _f32(np.asarray(inp["c2_b"], np.float32)[:, None])
    w["c3W"] = _bf16(inp["c3_W"])
    w["c3B"] = _f32(np.asarray(inp["c3_b"], np.float32)[:, None])
    return w


def _mask_ext(parity):
    """Slot-ordered additive mask encoding (slots 0-2 local, 3-5 remote)."""
    wk = np.concatenate([np.repeat(np.arange(16) * 2, C),
                         np.repeat(np.arange(16) * 2 + 1, C)])  # [672]
    wq = np.repeat(np.arange(16) * 2 + parity, C)               # [336]
    kext = np.zeros((6, 6, 112), np.float32)
    qext = np.zeros((6, 6, SL), np.float32)
    for s in range(6):
        g = 3 * parity + s if s < 3 else 3 * (1 - parity) + (s - 3)
        kwin = wk[g * 112:(g + 1) * 112]
        wins = sorted(set(kwin.tolist()))
        assert len(wins) == 6
        for r, wn in enumerate(wins):
            kext[r, s, :] = (kwin == wn)
            qext[r, s, :] = np.where(wn > wq, -30.0, 0.0)
    return _bf16(kext), _bf16(qext)


def kernel(**inputs):
    global _BUILT, LAST_RESULT
    if _BUILT is None:
        _BUILT = _build()
    nc, names = _BUILT

    w = _host_weights(inputs)
    x = np.asarray(inputs["x"], np.float32)  # [4, 3072, 21]

    kq_ext = [_mask_ext(p) for p in range(2)]
    in_maps = []
    for core in range(8):
        b, parity = core // 2, core % 2
        wins = np.arange(16) * 2 + parity
        xb = x[b]  # [3072, 21]
        xl = np.empty((P, SL), np.float32)
        for i, wn in enumerate(wins):
            xl[:, i * C:(i + 1) * C] = xb[wn * P:(wn + 1) * P, :]
        m = dict(w)
        m["xfull"] = _f32(xb.T)
        m["xloc"] = _bf16(xl)
        m["kextM"], m["qextM"] = kq_ext[parity]
        in_maps.append(m)

    res = run_bass_kernel_spmd(nc, in_maps, core_ids=list(range(8)))
    LAST_RESULT = res
    logits = np.stack(
        [res.results[2 * b]["out"].reshape(2).astype(np.float32) for b in range(B)])
    return logits
